# revision 3
# baseline (speedup 1.0000x reference)
"""Quantum multi-head attention TRN2 kernel (self-contained).

Problem: x(4,2048,1024); qp=cos(x+theta) per-head(16x64); q/k/v = qp@W*+b*
(per-head shared 64x64 weights); full softmax attention; merge heads; @Wo+bo.

Sharding: 8 cores = (batch b, seq-half j).  Each core gets the full batch-b
sequence (rolled so its 1024 query rows come first) and computes attention for
all 16 heads over its query rows, plus the final out-projection.  No
collectives; host just concatenates core outputs.

Device algorithm per core:
  qpT  = cos(xT+theta)  (E,S) bf16   - transposed layout, heads on partitions
  qpn  = cos(xn+theta)  [qp|1] tiles - natural layout + ones column
  kT/qT = W @ qpT       per head-pair, 2 heads packed via PE row/col tiling
  scoresT(j,i) = kT^T q  (2 heads concurrent via row tiling, K=64)
  e = exp(scores/8)     ACT over 2-bank PSUM tiles, bf16 out
  ctxT(d,i)+denom = [qp|1]^T @ e   accumulated over j in PSUM
  ctx = ctxT * (1/denom)           (DMA-broadcast reciprocal)
  out = ctx^T @ (blockdiag(Wv)@Wo) + (sum_h bv@Wo_h + bo)
"""
import numpy as np
import ml_dtypes

import concourse.bass as bass
import concourse.mybir as mybir
import concourse.tile as tile
from concourse.bass_utils import run_bass_kernel_spmd

F32 = mybir.dt.float32
F32R = mybir.dt.float32r
BF16 = mybir.dt.bfloat16
nbf16 = ml_dtypes.bfloat16
PI = float(np.pi)
MAGIC = 12582912.0  # 1.5 * 2**23 round-to-nearest magic
A = mybir.AluOpType
AF = mybir.ActivationFunctionType

B, S, E = 4, 2048, 1024
H, HD = 16, 64
SQ = 1024          # query rows per core
NPAIR = 8          # head pairs
N_CORES = 8
TRACE = False
LAST_RES = None


def _split_multiwaits(nc):
    """This container's walrus supports ONE sync-wait per instruction; split
    extras onto single-wait no-ops on the same engine (program order keeps
    semantics)."""
    counter = 0
    for f in nc.m.functions:
        for bb in f.blocks:
            new_insts = []
            for inst in bb.instructions:
                si = inst.sync_info
                if si is not None and si.on_wait and len(si.on_wait) > 1:
                    waits = list(si.on_wait)
                    si.on_wait = [waits[-1]]
                    for w in waits[:-1]:
                        counter += 1
                        new_insts.append(mybir.InstNoOp(
                            name=f"splitw-{counter}",
                            engine=inst.engine,
                            sync_info=mybir.SyncInfo(on_wait=[w], on_update=[]),
                            bass_nofuse=True,
                        ))
                new_insts.append(inst)
            bb.instructions[:] = new_insts
    return counter


def _build():
    nc = bass.Bass("TRN2", target_bir_lowering=False, debug=False)

    xt = nc.dram_tensor("xt", [E, S], F32, kind="ExternalInput")
    xn = nc.dram_tensor("xn", [S, E], F32, kind="ExternalInput")
    tht = nc.dram_tensor("tht", [128, 1], F32, kind="ExternalInput")
    thbc = nc.dram_tensor("thbc", [128, E], F32, kind="ExternalInput")
    wq2 = nc.dram_tensor("wq2", [128, HD], BF16, kind="ExternalInput")
    wk2 = nc.dram_tensor("wk2", [128, HD], BF16, kind="ExternalInput")
    wvt2 = nc.dram_tensor("wvt2", [128, HD], F32, kind="ExternalInput")
    wo = nc.dram_tensor("wo", [E, E], F32, kind="ExternalInput")
    bq2 = nc.dram_tensor("bq2", [128, 1], F32, kind="ExternalInput")
    bk2 = nc.dram_tensor("bk2", [128, 1], F32, kind="ExternalInput")
    bv2 = nc.dram_tensor("bv2", [128, 1], F32, kind="ExternalInput")
    bo_r = nc.dram_tensor("bo_r", [1, E], F32, kind="ExternalInput")
    out = nc.dram_tensor("out", [SQ, E], F32, kind="ExternalOutput")

    with tile.TileContext(nc) as tc:
        with tc.tile_pool(name="persist", bufs=1) as pp:
            # ---- persistent consts
            tht_t = pp.tile([128, 1], F32, name="tht_t")
            nc.sync.dma_start(tht_t[:], tht.ap())
            thbc_t = pp.tile([128, E], F32, name="thbc_t")
            nc.sync.dma_start(thbc_t[:], thbc.ap())
            wq2_t = pp.tile([128, HD], BF16, name="wq2_t")
            nc.sync.dma_start(wq2_t[:], wq2.ap())
            wk2_t = pp.tile([128, HD], BF16, name="wk2_t")
            nc.sync.dma_start(wk2_t[:], wk2.ap())
            wvt2_f = pp.tile([128, HD], F32, name="wvt2_f")
            nc.sync.dma_start(wvt2_f[:], wvt2.ap())
            wvt2_t = pp.tile([128, HD], F32R, name="wvt2_t")
            nc.vector.tensor_copy(wvt2_t[:], wvt2_f[:])
            bq2_t = pp.tile([128, 1], F32, name="bq2_t")
            nc.sync.dma_start(bq2_t[:], bq2.ap())
            bk2_t = pp.tile([128, 1], F32, name="bk2_t")
            nc.sync.dma_start(bk2_t[:], bk2.ap())
            bv2_f = pp.tile([128, 1], F32, name="bv2_f")
            nc.sync.dma_start(bv2_f[:], bv2.ap())
            bv2_t = pp.tile([128, 1], F32R, name="bv2_t")
            nc.vector.tensor_copy(bv2_t[:], bv2_f[:])
            borow_t = pp.tile([1, E], F32, name="borow_t")
            nc.sync.dma_start(borow_t[:], bo_r.ap())
            bobc_t = pp.tile([128, E], F32, name="bobc_t")

            # persistent big arrays
            qpT = [pp.tile([128, S], BF16, name=f"qpT_{t}") for t in range(8)]
            qpn = [pp.tile([128, H * 65], BF16, name=f"qpn_{j}") for j in range(16)]
            ctxT = [pp.tile([128, SQ], F32R, name=f"ctxT_{t}") for t in range(8)]
            wvo = [pp.tile([128, E], F32R, name=f"wvo_{t}") for t in range(8)]

            # ================= phase 0: Wvo = blockdiag(Wv) @ Wo, bvWo+bo ====
            with (
                tc.tile_pool(name="ph0", bufs=2) as p0,
                tc.tile_pool(name="ps0a", bufs=1, space="PSUM") as ps0a,
                tc.tile_pool(name="ps0b", bufs=2, space="PSUM") as ps0b,
                tc.tile_pool(name="dr0", bufs=1, space="DRAM") as dr0,
            ):
                bvwo_ps = ps0a.tile([1, E], F32, name="bvwo_ps")
                for t in range(8):
                    wo_f = p0.tile([128, E], F32, name=f"wof_{t}", tag="wo_inf")
                    nc.sync.dma_start(wo_f[:], wo.ap()[128 * t:128 * t + 128, :])
                    wo_t = p0.tile([128, E], F32R, name=f"wo_{t}", tag="wo_in")
                    nc.vector.tensor_copy(wo_t[:], wo_f[:])
                    for nt in range(2):
                        ns = slice(512 * nt, 512 * nt + 512)
                        # f32r matmuls cannot target col-offset 64, so use
                        # two 64-row psum tiles at base 0 (row tiling only)
                        wvpsA = ps0b.tile([64, 512], F32,
                                          name=f"wvpsA_{t}_{nt}", tag="wvopsA")
                        nc.tensor.matmul(wvpsA[:], wvt2_t[0:64, :],
                                         wo_t[0:64, ns], start=True, stop=True)
                        wvpsB = ps0b.tile([64, 512], F32,
                                          name=f"wvpsB_{t}_{nt}", tag="wvopsB")
                        nc.tensor.matmul(wvpsB[:], wvt2_t[64:128, :],
                                         wo_t[64:128, ns], start=True, stop=True)
                        nc.vector.tensor_copy(wvo[t][0:64, ns], wvpsA[:])
                        nc.vector.tensor_copy(wvo[t][64:128, ns], wvpsB[:])
                        nc.tensor.matmul(bvwo_ps[0:1, ns], bv2_t[:], wo_t[:, ns],
                                         start=(t == 0), stop=(t == 7))
                bosum_t = p0.tile([1, E], F32, name="bosum", tag="bosum")
                nc.vector.tensor_add(bosum_t[:], bvwo_ps[:], borow_t[:])
                bod = dr0.tile([1, E], F32, name="bod")
                nc.sync.dma_start(bod[:], bosum_t[:])
                nc.sync.dma_start(bobc_t[:], bod[:].broadcast_to([128, E]))

            # ================= phase 1: qpT / qpn (cos via Sin) ==============
            with tc.tile_pool(name="ph1", bufs=2) as p1:
                inv2pi = 1.0 / (2.0 * PI)
                for t in range(8):
                    xt_t = p1.tile([128, S], F32, name=f"xt_{t}", tag="xt_in")
                    nc.sync.dma_start(xt_t[:], xt.ap()[128 * t:128 * t + 128, :])
                    # u = x/(2pi) + th'   (th' = (theta+pi/2)/2pi per-partition)
                    nc.vector.tensor_scalar(xt_t[:], xt_t[:], inv2pi,
                                            tht_t[:, 0:1], A.mult, A.add)
                    rt = p1.tile([128, S], F32, name=f"rt_{t}", tag="rt")
                    nc.vector.tensor_scalar(rt[:], xt_t[:], MAGIC, MAGIC,
                                            A.add, A.subtract)
                    nc.vector.tensor_tensor(xt_t[:], xt_t[:], rt[:], A.subtract)
                    nc.scalar.activation(qpT[t][:], xt_t[:], AF.Sin,
                                         bias=0.0, scale=2.0 * PI)
                for jn in range(16):
                    xn_t = p1.tile([128, E], F32, name=f"xn_{jn}", tag="xn_in")
                    nc.sync.dma_start(xn_t[:], xn.ap()[128 * jn:128 * jn + 128, :])
                    nc.vector.scalar_tensor_tensor(xn_t[:], xn_t[:], inv2pi,
                                                   thbc_t[:], A.mult, A.add)
                    tn = p1.tile([128, E], F32, name=f"tn_{jn}", tag="tn")
                    nc.vector.tensor_scalar(tn[:], xn_t[:], MAGIC, MAGIC,
                                            A.add, A.subtract)
                    nc.vector.tensor_tensor(xn_t[:], xn_t[:], tn[:], A.subtract)
                    nc.vector.memset(qpn[jn][:], 1.0)
                    qv = qpn[jn][:].rearrange("p (h c) -> p h c", c=65)
                    uv = xn_t[:].rearrange("p (h c) -> p h c", c=64)
                    nc.scalar.activation(qv[:, :, 0:64], uv, AF.Sin,
                                         bias=0.0, scale=2.0 * PI)

            # ============ phase 2+3: projections + attention per pair ========
            with (
                tc.tile_pool(name="kq", bufs=2) as kq_pool,
                tc.tile_pool(name="et", bufs=2) as et_pool,
                tc.tile_pool(name="nrm", bufs=2) as nrm_pool,
                tc.tile_pool(name="drb", bufs=4, space="DRAM") as dr_pool,
                tc.tile_pool(name="ps_s", bufs=2, space="PSUM") as ps_s,
                tc.tile_pool(name="ps_c", bufs=2, space="PSUM") as ps_c,
                tc.tile_pool(name="ps_p", bufs=2, space="PSUM") as ps_p,
            ):
                for t in range(8):
                    hA, hB = 2 * t, 2 * t + 1
                    kT = kq_pool.tile([128, S], BF16, name=f"kT_{t}", tag="kT")
                    qT = kq_pool.tile([128, SQ], BF16, name=f"qT_{t}", tag="qT")
                    for st in range(4):
                        ss = slice(512 * st, 512 * st + 512)
                        pps = ps_p.tile([128, 512], F32,
                                        name=f"kps_{t}_{st}", tag="pps")
                        nc.tensor.matmul(pps[0:64, :], wk2_t[0:64, :],
                                         qpT[t][0:64, ss], start=True, stop=True)
                        nc.tensor.matmul(pps[64:128, :], wk2_t[64:128, :],
                                         qpT[t][64:128, ss], start=True, stop=True)
                        nc.vector.tensor_scalar_add(kT[:, ss], pps[:],
                                                    bk2_t[:, 0:1])
                    for st in range(2):
                        ss = slice(512 * st, 512 * st + 512)
                        pps = ps_p.tile([128, 512], F32,
                                        name=f"qps_{t}_{st}", tag="pps")
                        nc.tensor.matmul(pps[0:64, :], wq2_t[0:64, :],
                                         qpT[t][0:64, ss], start=True, stop=True)
                        nc.tensor.matmul(pps[64:128, :], wq2_t[64:128, :],
                                         qpT[t][64:128, ss], start=True, stop=True)
                        nc.vector.tensor_scalar_add(qT[:, ss], pps[:],
                                                    bq2_t[:, 0:1])

                    for it in range(2):
                        isl = slice(512 * it, 512 * it + 512)
                        cA = ps_c.tile([65, 512], F32,
                                       name=f"cA_{t}_{it}", tag="ctx")
                        cB = ps_c.tile([65, 512], F32,
                                       name=f"cB_{t}_{it}", tag="ctx")
                        for j2 in range(8):
                            sA = ps_s.tile([128, 1024], F32,
                                           name=f"sA_{t}_{it}_{j2}", tag="spair")
                            sB = ps_s.tile([128, 1024], F32,
                                           name=f"sB_{t}_{it}_{j2}", tag="spair")
                            for hf in range(2):
                                jc = 2 * j2 + hf
                                js = slice(128 * jc, 128 * jc + 128)
                                hs = slice(512 * hf, 512 * hf + 512)
                                nc.tensor.matmul(sA[:, hs], kT[0:64, js],
                                                 qT[0:64, isl],
                                                 start=True, stop=True)
                                nc.tensor.matmul(sB[:, hs], kT[64:128, js],
                                                 qT[64:128, isl],
                                                 start=True, stop=True)
                            eA = et_pool.tile([128, 1024], BF16,
                                              name=f"eA_{t}_{it}_{j2}", tag="eA")
                            nc.scalar.activation(eA[:], sA[:], AF.Exp,
                                                 bias=0.0, scale=0.125)
                            eB = et_pool.tile([128, 1024], BF16,
                                              name=f"eB_{t}_{it}_{j2}", tag="eB")
                            nc.scalar.activation(eB[:], sB[:], AF.Exp,
                                                 bias=0.0, scale=0.125)
                            for hf in range(2):
                                jc = 2 * j2 + hf
                                hs = slice(512 * hf, 512 * hf + 512)
                                st_ = (j2 == 0 and hf == 0)
                                sp_ = (j2 == 7 and hf == 1)
                                nc.tensor.matmul(
                                    cA[:], qpn[jc][:, 65 * hA:65 * hA + 65],
                                    eA[:, hs], start=st_, stop=sp_)
                                nc.tensor.matmul(
                                    cB[:], qpn[jc][:, 65 * hB:65 * hB + 65],
                                    eB[:, hs], start=st_, stop=sp_)
                        for head, cps in ((0, cA), (1, cB)):
                            rc = nrm_pool.tile([1, 512], F32,
                                               name=f"rc_{t}_{it}_{head}",
                                               tag="rc")
                            nc.vector.reciprocal(rc[:], cps[64:65, :])
                            dr = dr_pool.tile([1, 512], F32,
                                              name=f"dr_{t}_{it}_{head}",
                                              tag="dr")
                            nc.sync.dma_start(dr[:], rc[:])
                            bc = nrm_pool.tile([64, 512], F32,
                                               name=f"bc_{t}_{it}_{head}",
                                               tag="bc")
                            nc.sync.dma_start(bc[:], dr[:].broadcast_to([64, 512]))
                            nc.vector.tensor_mul(
                                ctxT[t][64 * head:64 * head + 64, isl],
                                cps[0:64, :], bc[:])

            # ================= phase 4: out projection =======================
            with (
                tc.tile_pool(name="ph4", bufs=2) as p4,
                tc.tile_pool(name="ps4", bufs=2, space="PSUM") as ps4,
            ):
                for ic in range(8):
                    ics = slice(128 * ic, 128 * ic + 128)
                    ot = p4.tile([128, E], F32, name=f"ot_{ic}", tag="ot")
                    for nt in range(2):
                        ns = slice(512 * nt, 512 * nt + 512)
                        ops_ = ps4.tile([128, 512], F32,
                                        name=f"ops_{ic}_{nt}", tag="ops")
                        for t in range(8):
                            nc.tensor.matmul(ops_[:], ctxT[t][:, ics],
                                             wvo[t][:, ns],
                                             start=(t == 0), stop=(t == 7))
                        nc.vector.tensor_add(ot[:, ns], ops_[:], bobc_t[:, ns])
                    nc.sync.dma_start(out.ap()[ics, :], ot[:])

    return nc


def kernel(x, theta, Wq, bq, Wk, bk, Wv, bv, Wo, bo):
    x = np.asarray(x, np.float32)
    theta = np.asarray(theta, np.float32)
    Wq = np.asarray(Wq, np.float32)
    Wk = np.asarray(Wk, np.float32)
    Wv = np.asarray(Wv, np.float32)
    Wo = np.asarray(Wo, np.float32)
    bq = np.asarray(bq, np.float32)
    bk = np.asarray(bk, np.float32)
    bv = np.asarray(bv, np.float32)
    bo = np.asarray(bo, np.float32)

    nc = _build()
    _split_multiwaits(nc)

    th2 = np.concatenate([theta, theta]).reshape(128, 1)
    tht = ((th2 + PI / 2) / (2 * PI)).astype(np.float32)
    thbc = np.tile(
        ((np.tile(theta, H) + PI / 2) / (2 * PI)).astype(np.float32).reshape(1, E),
        (128, 1),
    )
    wq2 = np.concatenate([Wq, Wq], axis=0).astype(nbf16)
    wk2 = np.concatenate([Wk, Wk], axis=0).astype(nbf16)
    wvt2 = np.ascontiguousarray(np.concatenate([Wv.T, Wv.T], axis=0), dtype=np.float32)
    wo_bf = np.ascontiguousarray(Wo, dtype=np.float32)
    bq2 = np.concatenate([bq, bq]).reshape(128, 1).astype(np.float32)
    bk2 = np.concatenate([bk, bk]).reshape(128, 1).astype(np.float32)
    bv2 = np.concatenate([bv, bv]).reshape(128, 1).astype(np.float32)
    bo_r = bo.reshape(1, E).astype(np.float32)

    in_maps = []
    for c in range(N_CORES):
        b, j = c // 2, c % 2
        xb = np.roll(x[b], -SQ * j, axis=0)
        in_maps.append(dict(
            xt=np.ascontiguousarray(xb.T),
            xn=np.ascontiguousarray(xb),
            tht=tht, thbc=thbc, wq2=wq2, wk2=wk2, wvt2=wvt2, wo=wo_bf,
            bq2=bq2, bk2=bk2, bv2=bv2, bo_r=bo_r,
        ))

    kw = {}
    if TRACE:
        kw = dict(trace=True, trace_cores=[0])
    res = run_bass_kernel_spmd(nc, in_maps, core_ids=list(range(N_CORES)), **kw)
    global LAST_RES
    LAST_RES = res

    out = np.empty((B, S, E), np.float32)
    for c in range(N_CORES):
        b, j = c // 2, c % 2
        out[b, SQ * j:SQ * (j + 1), :] = res.results[c]["out"]
    return out


# revision 8
# speedup vs baseline: 2.0280x; 2.0280x over previous
"""Quantum multi-head attention TRN2 kernel (self-contained).

Problem: x(4,2048,1024); qp=cos(x+theta) per-head(16x64); q/k/v = qp@W*+b*
(per-head shared 64x64 weights); full softmax attention; merge heads; @Wo+bo.

Sharding: 8 cores = (batch b, seq-half j).  Each core gets the full batch-b
sequence (rolled so its 1024 query rows come first) and computes attention for
all 16 heads over its query rows, plus the final out-projection.  No
collectives; host just concatenates core outputs.

Device algorithm per core:
  qpT  = cos(xT+theta)  (E,S) bf16   - transposed layout, heads on partitions
  qpn  = cos(xn+theta)  [qp|1] tiles - natural layout + ones column
  kT/qT = W @ qpT       per head-pair, 2 heads packed via PE row/col tiling
  scoresT(j,i) = kT^T q  (2 heads concurrent via row tiling, K=64)
  e = exp(scores/8)     ACT over 2-bank PSUM tiles, bf16 out
  ctxT(d,i)+denom = [qp|1]^T @ e   accumulated over j in PSUM
  ctx = ctxT * (1/denom)           (DMA-broadcast reciprocal)
  out = ctx^T @ (blockdiag(Wv)@Wo) + (sum_h bv@Wo_h + bo)
"""
import numpy as np
import ml_dtypes

import concourse.bass as bass
import concourse.mybir as mybir
import concourse.tile as tile
from concourse.bass_utils import run_bass_kernel_spmd

F32 = mybir.dt.float32
F32R = mybir.dt.float32r
BF16 = mybir.dt.bfloat16
nbf16 = ml_dtypes.bfloat16
PI = float(np.pi)
MAGIC = 12582912.0  # 1.5 * 2**23 round-to-nearest magic
A = mybir.AluOpType
AF = mybir.ActivationFunctionType

B, S, E = 4, 2048, 1024
H, HD = 16, 64
SQ = 1024          # query rows per core
NPAIR = 8          # head pairs
N_CORES = 8
TRACE = False
LAST_RES = None


def _split_multiwaits(nc):
    """This container's walrus supports ONE sync-wait per instruction; split
    extras onto single-wait no-ops on the same engine (program order keeps
    semantics)."""
    counter = 0
    for f in nc.m.functions:
        for bb in f.blocks:
            new_insts = []
            for inst in bb.instructions:
                si = inst.sync_info
                if si is not None and si.on_wait and len(si.on_wait) > 1:
                    waits = list(si.on_wait)
                    si.on_wait = [waits[-1]]
                    for w in waits[:-1]:
                        counter += 1
                        new_insts.append(mybir.InstNoOp(
                            name=f"splitw-{counter}",
                            engine=inst.engine,
                            sync_info=mybir.SyncInfo(on_wait=[w], on_update=[]),
                            bass_nofuse=True,
                        ))
                new_insts.append(inst)
            bb.instructions[:] = new_insts
    return counter


def _build(phases=4, lite=False, attn_reps=1, p0_reps=1, p1_reps=1, p4_reps=1):
    nc = bass.Bass("TRN2", target_bir_lowering=False, debug=False)

    big = "Internal" if lite else "ExternalInput"
    xt = nc.dram_tensor("xt", [E, S], F32, kind=big)
    xn = nc.dram_tensor("xn", [S, E], F32, kind=big)
    tht = nc.dram_tensor("tht", [128, 1], F32, kind="ExternalInput")
    thbc = nc.dram_tensor("thbc", [128, E], F32, kind="ExternalInput")
    wq2 = nc.dram_tensor("wq2", [128, HD], BF16, kind="ExternalInput")
    wk2 = nc.dram_tensor("wk2", [128, HD], BF16, kind="ExternalInput")
    wvt2 = nc.dram_tensor("wvt2", [128, HD], F32, kind="ExternalInput")
    wo = nc.dram_tensor("wo", [E, E], F32, kind=big)
    bq2 = nc.dram_tensor("bq2", [128, 1], F32, kind="ExternalInput")
    bk2 = nc.dram_tensor("bk2", [128, 1], F32, kind="ExternalInput")
    bv2 = nc.dram_tensor("bv2", [128, 1], F32, kind="ExternalInput")
    bo_r = nc.dram_tensor("bo_r", [1, E], F32, kind="ExternalInput")
    out = nc.dram_tensor("out", [SQ, E], F32, kind="ExternalOutput")

    with tile.TileContext(nc) as tc:
        with tc.tile_pool(name="persist", bufs=1) as pp:
            # ---- persistent consts
            tht_t = pp.tile([128, 1], F32, name="tht_t")
            nc.sync.dma_start(tht_t[:], tht.ap())
            thbc_t = pp.tile([128, E], F32, name="thbc_t")
            nc.sync.dma_start(thbc_t[:], thbc.ap())
            wq2_t = pp.tile([128, HD], BF16, name="wq2_t")
            nc.sync.dma_start(wq2_t[:], wq2.ap())
            wk2_t = pp.tile([128, HD], BF16, name="wk2_t")
            nc.sync.dma_start(wk2_t[:], wk2.ap())
            wvt2_f = pp.tile([128, HD], F32, name="wvt2_f")
            nc.sync.dma_start(wvt2_f[:], wvt2.ap())
            wvt2_t = pp.tile([128, HD], F32R, name="wvt2_t")
            nc.vector.tensor_copy(wvt2_t[:], wvt2_f[:])
            bq2_t = pp.tile([128, 1], F32, name="bq2_t")
            nc.sync.dma_start(bq2_t[:], bq2.ap())
            bk2_t = pp.tile([128, 1], F32, name="bk2_t")
            nc.sync.dma_start(bk2_t[:], bk2.ap())
            bv2_f = pp.tile([128, 1], F32, name="bv2_f")
            nc.sync.dma_start(bv2_f[:], bv2.ap())
            bv2_t = pp.tile([128, 1], F32R, name="bv2_t")
            nc.vector.tensor_copy(bv2_t[:], bv2_f[:])
            borow_t = pp.tile([1, E], F32, name="borow_t")
            nc.sync.dma_start(borow_t[:], bo_r.ap())
            bobc_t = pp.tile([128, E], F32, name="bobc_t")

            # persistent big arrays
            qpT = [pp.tile([128, S], BF16, name=f"qpT_{t}") for t in range(8)]
            qpn = [pp.tile([128, H * 65], BF16, name=f"qpn_{j}") for j in range(16)]
            ctxT = [pp.tile([128, SQ], F32R, name=f"ctxT_{t}") for t in range(8)]
            wvo = [pp.tile([128, E], F32R, name=f"wvo_{t}") for t in range(8)]

            # lite timing mode: zero the Internal scratch so exp() sees
            # sane values (NaN/Inf notifications would distort timing)
            if lite:
                with tc.tile_pool(name="zf", bufs=2) as zf:
                    zt = zf.tile([128, S], F32, name="zt", tag="zt")
                    nc.vector.memset(zt[:], 0.0)
                    for t in range(8):
                        nc.sync.dma_start(xt.ap()[128 * t:128 * t + 128, :], zt[:])
                    for jn in range(16):
                        nc.sync.dma_start(xn.ap()[128 * jn:128 * jn + 128, :],
                                          zt[:, 0:E])
                    for t in range(8):
                        nc.sync.dma_start(wo.ap()[128 * t:128 * t + 128, :],
                                          zt[:, 0:E])

            # ================= phase 0: Wvo = blockdiag(Wv) @ Wo, bvWo+bo ====
            if phases >= 0:
              with (
                tc.tile_pool(name="ph0", bufs=2) as p0,
                tc.tile_pool(name="ps0a", bufs=1, space="PSUM") as ps0a,
                tc.tile_pool(name="ps0b", bufs=2, space="PSUM") as ps0b,
                tc.tile_pool(name="dr0", bufs=1, space="DRAM") as dr0,
            ):
               for rep in range(p0_reps):
                bvwo_ps = ps0a.tile([1, E], F32, name=f"bvwo_ps_{rep}", tag="bvwo")
                for t in range(8):
                    wo_f = p0.tile([128, E], F32, name=f"wof_{rep}_{t}", tag="wo_inf")
                    nc.sync.dma_start(wo_f[:], wo.ap()[128 * t:128 * t + 128, :])
                    wo_t = p0.tile([128, E], F32R, name=f"wo_{rep}_{t}", tag="wo_in")
                    nc.vector.tensor_copy(wo_t[:], wo_f[:])
                    for nt in range(2):
                        ns = slice(512 * nt, 512 * nt + 512)
                        # f32r matmuls cannot target col-offset 64, so use
                        # two 64-row psum tiles at base 0 (row tiling only)
                        wvpsA = ps0b.tile([64, 512], F32,
                                          name=f"wvpsA_{rep}_{t}_{nt}", tag="wvopsA")
                        nc.tensor.matmul(wvpsA[:], wvt2_t[0:64, :],
                                         wo_t[0:64, ns], start=True, stop=True)
                        wvpsB = ps0b.tile([64, 512], F32,
                                          name=f"wvpsB_{rep}_{t}_{nt}", tag="wvopsB")
                        nc.tensor.matmul(wvpsB[:], wvt2_t[64:128, :],
                                         wo_t[64:128, ns], start=True, stop=True)
                        nc.vector.tensor_copy(wvo[t][0:64, ns], wvpsA[:])
                        nc.vector.tensor_copy(wvo[t][64:128, ns], wvpsB[:])
                        nc.tensor.matmul(bvwo_ps[0:1, ns], bv2_t[:], wo_t[:, ns],
                                         start=(t == 0), stop=(t == 7))
                bosum_t = p0.tile([1, E], F32, name=f"bosum_{rep}", tag="bosum")
                nc.vector.tensor_add(bosum_t[:], bvwo_ps[:], borow_t[:])
                bod = dr0.tile([1, E], F32, name=f"bod_{rep}", tag="bod")
                nc.sync.dma_start(bod[:], bosum_t[:])
                nc.sync.dma_start(bobc_t[:], bod[:].broadcast_to([128, E]))

            # ================= phase 1: qpT / qpn (cos via Sin) ==============
            if phases >= 1:
              with tc.tile_pool(name="ph1", bufs=2) as p1:
               for rep in range(p1_reps):
                inv2pi = 1.0 / (2.0 * PI)
                for t in range(8):
                    xt_t = p1.tile([128, S], F32, name=f"xt_{rep}_{t}", tag="xt_in")
                    nc.sync.dma_start(xt_t[:], xt.ap()[128 * t:128 * t + 128, :])
                    # u = x/(2pi) + th'   (th' = (theta+pi/2)/2pi per-partition)
                    nc.vector.tensor_scalar(xt_t[:], xt_t[:], inv2pi,
                                            tht_t[:, 0:1], A.mult, A.add)
                    rt = p1.tile([128, S], F32, name=f"rt_{rep}_{t}", tag="rt")
                    nc.vector.tensor_scalar(rt[:], xt_t[:], MAGIC, MAGIC,
                                            A.add, A.subtract)
                    nc.vector.tensor_tensor(xt_t[:], xt_t[:], rt[:], A.subtract)
                    nc.scalar.activation(qpT[t][:], xt_t[:], AF.Sin,
                                         bias=0.0, scale=2.0 * PI)
                for jn in range(16):
                    xn_t = p1.tile([128, E], F32, name=f"xn_{rep}_{jn}", tag="xn_in")
                    nc.sync.dma_start(xn_t[:], xn.ap()[128 * jn:128 * jn + 128, :])
                    nc.vector.scalar_tensor_tensor(xn_t[:], xn_t[:], inv2pi,
                                                   thbc_t[:], A.mult, A.add)
                    tn = p1.tile([128, E], F32, name=f"tn_{rep}_{jn}", tag="tn")
                    nc.vector.tensor_scalar(tn[:], xn_t[:], MAGIC, MAGIC,
                                            A.add, A.subtract)
                    nc.vector.tensor_tensor(xn_t[:], xn_t[:], tn[:], A.subtract)
                    nc.vector.memset(qpn[jn][:], 1.0)
                    qv = qpn[jn][:].rearrange("p (h c) -> p h c", c=65)
                    uv = xn_t[:].rearrange("p (h c) -> p h c", c=64)
                    nc.scalar.activation(qv[:, :, 0:64], uv, AF.Sin,
                                         bias=0.0, scale=2.0 * PI)

            # ============ phase 2+3: projections + attention per pair ========
            if phases >= 2:
              with (
                tc.tile_pool(name="kq", bufs=2) as kq_pool,
                tc.tile_pool(name="et", bufs=2) as et_pool,
                tc.tile_pool(name="nrm", bufs=2) as nrm_pool,
                tc.tile_pool(name="drb", bufs=4, space="DRAM") as dr_pool,
                tc.tile_pool(name="ps_s", bufs=2, space="PSUM") as ps_s,
                tc.tile_pool(name="ps_c", bufs=2, space="PSUM") as ps_c,
                tc.tile_pool(name="ps_p", bufs=2, space="PSUM") as ps_p,
            ):
               for rep in range(attn_reps):
                for t in range(8):
                    hA, hB = 2 * t, 2 * t + 1
                    kT = kq_pool.tile([128, S], BF16, name=f"kT_{rep}_{t}", tag="kT")
                    qT = kq_pool.tile([128, SQ], BF16, name=f"qT_{rep}_{t}", tag="qT")
                    for st in range(4):
                        ss = slice(512 * st, 512 * st + 512)
                        pps = ps_p.tile([128, 512], F32,
                                        name=f"kps_{rep}_{t}_{st}", tag="pps")
                        nc.tensor.matmul(pps[0:64, :], wk2_t[0:64, :],
                                         qpT[t][0:64, ss], start=True, stop=True)
                        nc.tensor.matmul(pps[64:128, :], wk2_t[64:128, :],
                                         qpT[t][64:128, ss], start=True, stop=True)
                        nc.vector.tensor_scalar_add(kT[:, ss], pps[:],
                                                    bk2_t[:, 0:1])
                    for st in range(2):
                        ss = slice(512 * st, 512 * st + 512)
                        pps = ps_p.tile([128, 512], F32,
                                        name=f"qps_{rep}_{t}_{st}", tag="pps")
                        nc.tensor.matmul(pps[0:64, :], wq2_t[0:64, :],
                                         qpT[t][0:64, ss], start=True, stop=True)
                        nc.tensor.matmul(pps[64:128, :], wq2_t[64:128, :],
                                         qpT[t][64:128, ss], start=True, stop=True)
                        nc.vector.tensor_scalar_add(qT[:, ss], pps[:],
                                                    bq2_t[:, 0:1])

                    for it in range(2):
                        isl = slice(512 * it, 512 * it + 512)
                        cA = ps_c.tile([65, 512], F32,
                                       name=f"cA_{rep}_{t}_{it}", tag="ctx")
                        cB = ps_c.tile([65, 512], F32,
                                       name=f"cB_{rep}_{t}_{it}", tag="ctx")
                        for j2 in range(8):
                            sA = ps_s.tile([128, 1024], F32,
                                           name=f"sA_{rep}_{t}_{it}_{j2}", tag="spair")
                            sB = ps_s.tile([128, 1024], F32,
                                           name=f"sB_{rep}_{t}_{it}_{j2}", tag="spair")
                            for hf in range(2):
                                jc = 2 * j2 + hf
                                js = slice(128 * jc, 128 * jc + 128)
                                hs = slice(512 * hf, 512 * hf + 512)
                                nc.tensor.matmul(sA[:, hs], kT[0:64, js],
                                                 qT[0:64, isl],
                                                 start=True, stop=True)
                                nc.tensor.matmul(sB[:, hs], kT[64:128, js],
                                                 qT[64:128, isl],
                                                 start=True, stop=True)
                            eA = et_pool.tile([128, 1024], BF16,
                                              name=f"eA_{rep}_{t}_{it}_{j2}", tag="eA")
                            nc.scalar.activation(eA[:], sA[:], AF.Exp,
                                                 bias=0.0, scale=0.125)
                            eB = et_pool.tile([128, 1024], BF16,
                                              name=f"eB_{rep}_{t}_{it}_{j2}", tag="eB")
                            nc.scalar.activation(eB[:], sB[:], AF.Exp,
                                                 bias=0.0, scale=0.125)
                            for hf in range(2):
                                jc = 2 * j2 + hf
                                hs = slice(512 * hf, 512 * hf + 512)
                                st_ = (j2 == 0 and hf == 0)
                                sp_ = (j2 == 7 and hf == 1)
                                nc.tensor.matmul(
                                    cA[:], qpn[jc][:, 65 * hA:65 * hA + 65],
                                    eA[:, hs], start=st_, stop=sp_)
                                nc.tensor.matmul(
                                    cB[:], qpn[jc][:, 65 * hB:65 * hB + 65],
                                    eB[:, hs], start=st_, stop=sp_)
                        for head, cps in ((0, cA), (1, cB)):
                            rc = nrm_pool.tile([1, 512], F32,
                                               name=f"rc_{rep}_{t}_{it}_{head}",
                                               tag="rc")
                            nc.vector.reciprocal(rc[:], cps[64:65, :])
                            dr = dr_pool.tile([1, 512], F32,
                                              name=f"dr_{rep}_{t}_{it}_{head}",
                                              tag="dr")
                            nc.sync.dma_start(dr[:], rc[:])
                            bc = nrm_pool.tile([64, 512], F32,
                                               name=f"bc_{rep}_{t}_{it}_{head}",
                                               tag="bc")
                            nc.sync.dma_start(bc[:], dr[:].broadcast_to([64, 512]))
                            nc.vector.tensor_mul(
                                ctxT[t][64 * head:64 * head + 64, isl],
                                cps[0:64, :], bc[:])

            # ================= phase 4: out projection =======================
            if phases >= 4:
              with (
                tc.tile_pool(name="ph4", bufs=2) as p4,
                tc.tile_pool(name="ps4", bufs=2, space="PSUM") as ps4,
            ):
               for rep in range(p4_reps):
                for ic in range(8):
                    ics = slice(128 * ic, 128 * ic + 128)
                    ot = p4.tile([128, E], F32, name=f"ot_{rep}_{ic}", tag="ot")
                    for nt in range(2):
                        ns = slice(512 * nt, 512 * nt + 512)
                        ops_ = ps4.tile([128, 512], F32,
                                        name=f"ops_{rep}_{ic}_{nt}", tag="ops")
                        for t in range(8):
                            nc.tensor.matmul(ops_[:], ctxT[t][:, ics],
                                             wvo[t][:, ns],
                                             start=(t == 0), stop=(t == 7))
                        nc.vector.tensor_add(ot[:, ns], ops_[:], bobc_t[:, ns])
                    nc.sync.dma_start(out.ap()[ics, :], ot[:])

    return nc


def kernel(x, theta, Wq, bq, Wk, bk, Wv, bv, Wo, bo):
    x = np.asarray(x, np.float32)
    theta = np.asarray(theta, np.float32)
    Wq = np.asarray(Wq, np.float32)
    Wk = np.asarray(Wk, np.float32)
    Wv = np.asarray(Wv, np.float32)
    Wo = np.asarray(Wo, np.float32)
    bq = np.asarray(bq, np.float32)
    bk = np.asarray(bk, np.float32)
    bv = np.asarray(bv, np.float32)
    bo = np.asarray(bo, np.float32)

    nc = _build()
    _split_multiwaits(nc)

    th2 = np.concatenate([theta, theta]).reshape(128, 1)
    tht = ((th2 + PI / 2) / (2 * PI)).astype(np.float32)
    thbc = np.tile(
        ((np.tile(theta, H) + PI / 2) / (2 * PI)).astype(np.float32).reshape(1, E),
        (128, 1),
    )
    wq2 = np.concatenate([Wq, Wq], axis=0).astype(nbf16)
    wk2 = np.concatenate([Wk, Wk], axis=0).astype(nbf16)
    wvt2 = np.ascontiguousarray(np.concatenate([Wv.T, Wv.T], axis=0), dtype=np.float32)
    wo_bf = np.ascontiguousarray(Wo, dtype=np.float32)
    bq2 = np.concatenate([bq, bq]).reshape(128, 1).astype(np.float32)
    bk2 = np.concatenate([bk, bk]).reshape(128, 1).astype(np.float32)
    bv2 = np.concatenate([bv, bv]).reshape(128, 1).astype(np.float32)
    bo_r = bo.reshape(1, E).astype(np.float32)

    in_maps = []
    for c in range(N_CORES):
        b, j = c // 2, c % 2
        xb = np.roll(x[b], -SQ * j, axis=0)
        in_maps.append(dict(
            xt=np.ascontiguousarray(xb.T),
            xn=np.ascontiguousarray(xb),
            tht=tht, thbc=thbc, wq2=wq2, wk2=wk2, wvt2=wvt2, wo=wo_bf,
            bq2=bq2, bk2=bk2, bv2=bv2, bo_r=bo_r,
        ))

    kw = {}
    if TRACE:
        kw = dict(trace=True, trace_cores=[0])
    res = run_bass_kernel_spmd(nc, in_maps, core_ids=list(range(N_CORES)), **kw)
    global LAST_RES
    LAST_RES = res

    out = np.empty((B, S, E), np.float32)
    for c in range(N_CORES):
        b, j = c // 2, c % 2
        out[b, SQ * j:SQ * (j + 1), :] = res.results[c]["out"]
    return out


# revision 10
# speedup vs baseline: 2.5073x; 1.2363x over previous
"""Quantum multi-head attention TRN2 kernel (self-contained).

Problem: x(4,2048,1024); qp=cos(x+theta) per-head(16x64); q/k/v = qp@W*+b*
(per-head shared 64x64 weights); full softmax attention; merge heads; @Wo+bo.

Sharding: 8 cores = (batch b, seq-half j).  Each core gets the full batch-b
sequence (rolled so its 1024 query rows come first) and computes attention for
all 16 heads over its query rows, plus the final out-projection.  No
collectives; host just concatenates core outputs.

Device algorithm per core:
  qpT  = cos(xT+theta)  (E,S) bf16   - transposed layout, heads on partitions
  qpn  = cos(xn+theta)  [qp|1] tiles - natural layout + ones column
  kT/qT = W @ qpT       per head-pair, 2 heads packed via PE row/col tiling
  scoresT(j,i) = kT^T q  (2 heads concurrent via row tiling, K=64)
  e = exp(scores/8)     ACT over 2-bank PSUM tiles, bf16 out
  ctxT(d,i)+denom = [qp|1]^T @ e   accumulated over j in PSUM
  ctx = ctxT * (1/denom)           (DMA-broadcast reciprocal)
  out = ctx^T @ (blockdiag(Wv)@Wo) + (sum_h bv@Wo_h + bo)
"""
import numpy as np
import ml_dtypes

import concourse.bass as bass
import concourse.mybir as mybir
import concourse.tile as tile
from concourse.bass_utils import run_bass_kernel_spmd

F32 = mybir.dt.float32
F32R = mybir.dt.float32r
BF16 = mybir.dt.bfloat16
nbf16 = ml_dtypes.bfloat16
PI = float(np.pi)
MAGIC = 12582912.0  # 1.5 * 2**23 round-to-nearest magic
A = mybir.AluOpType
AF = mybir.ActivationFunctionType

B, S, E = 4, 2048, 1024
H, HD = 16, 64
SQ = 1024          # query rows per core
NPAIR = 8          # head pairs
N_CORES = 8
TRACE = False
LAST_RES = None


def _split_multiwaits(nc):
    """This container's walrus supports ONE sync-wait per instruction; split
    extras onto single-wait no-ops on the same engine (program order keeps
    semantics)."""
    counter = 0
    for f in nc.m.functions:
        for bb in f.blocks:
            new_insts = []
            for inst in bb.instructions:
                si = inst.sync_info
                if si is not None and si.on_wait and len(si.on_wait) > 1:
                    waits = list(si.on_wait)
                    si.on_wait = [waits[-1]]
                    for w in waits[:-1]:
                        counter += 1
                        new_insts.append(mybir.InstNoOp(
                            name=f"splitw-{counter}",
                            engine=inst.engine,
                            sync_info=mybir.SyncInfo(on_wait=[w], on_update=[]),
                            bass_nofuse=True,
                        ))
                new_insts.append(inst)
            bb.instructions[:] = new_insts
    return counter


def _build(phases=4, lite=False, attn_reps=1, p0_reps=1, p1_reps=1, p4_reps=1):
    nc = bass.Bass("TRN2", target_bir_lowering=False, debug=False)

    big = "Internal" if lite else "ExternalInput"
    xt = nc.dram_tensor("xt", [E, S], F32, kind=big)
    xn = nc.dram_tensor("xn", [S, E], F32, kind=big)
    tht = nc.dram_tensor("tht", [128, 1], F32, kind="ExternalInput")
    thbc = nc.dram_tensor("thbc", [128, E], F32, kind="ExternalInput")
    wq2 = nc.dram_tensor("wq2", [128, HD], BF16, kind="ExternalInput")
    wk2 = nc.dram_tensor("wk2", [128, HD], BF16, kind="ExternalInput")
    wvt2 = nc.dram_tensor("wvt2", [128, HD], F32, kind="ExternalInput")
    wo = nc.dram_tensor("wo", [E, E], F32, kind=big)
    bq2 = nc.dram_tensor("bq2", [128, 1], F32, kind="ExternalInput")
    bk2 = nc.dram_tensor("bk2", [128, 1], F32, kind="ExternalInput")
    bv2 = nc.dram_tensor("bv2", [128, 1], F32, kind="ExternalInput")
    bo_r = nc.dram_tensor("bo_r", [1, E], F32, kind="ExternalInput")
    out = nc.dram_tensor("out", [SQ, E], F32, kind="ExternalOutput")

    with tile.TileContext(nc) as tc:
        with tc.tile_pool(name="persist", bufs=1) as pp:
            # ---- persistent consts
            tht_t = pp.tile([128, 1], F32, name="tht_t")
            nc.sync.dma_start(tht_t[:], tht.ap())
            thbc_t = pp.tile([128, E], F32, name="thbc_t")
            nc.sync.dma_start(thbc_t[:], thbc.ap())
            wq2_t = pp.tile([128, HD], BF16, name="wq2_t")
            nc.sync.dma_start(wq2_t[:], wq2.ap())
            wk2_t = pp.tile([128, HD], BF16, name="wk2_t")
            nc.sync.dma_start(wk2_t[:], wk2.ap())
            wvt2_f = pp.tile([128, HD], F32, name="wvt2_f")
            nc.sync.dma_start(wvt2_f[:], wvt2.ap())
            wvt2_t = pp.tile([128, HD], F32R, name="wvt2_t")
            nc.vector.tensor_copy(wvt2_t[:], wvt2_f[:])
            bq2_t = pp.tile([128, 1], F32, name="bq2_t")
            nc.sync.dma_start(bq2_t[:], bq2.ap())
            bk2_t = pp.tile([128, 1], F32, name="bk2_t")
            nc.sync.dma_start(bk2_t[:], bk2.ap())
            bv2_f = pp.tile([128, 1], F32, name="bv2_f")
            nc.sync.dma_start(bv2_f[:], bv2.ap())
            bv2_t = pp.tile([128, 1], F32R, name="bv2_t")
            nc.vector.tensor_copy(bv2_t[:], bv2_f[:])
            borow_t = pp.tile([1, E], F32, name="borow_t")
            nc.sync.dma_start(borow_t[:], bo_r.ap())
            bobc_t = pp.tile([128, E], F32, name="bobc_t")

            # persistent big arrays
            qpT = [pp.tile([128, S], BF16, name=f"qpT_{t}") for t in range(8)]
            qpn = [pp.tile([128, H * 65], BF16, name=f"qpn_{j}") for j in range(16)]
            ctxT = [pp.tile([128, SQ], F32R, name=f"ctxT_{t}") for t in range(8)]
            wvo = [pp.tile([128, E], F32R, name=f"wvo_{t}") for t in range(8)]

            # lite timing mode: zero the Internal scratch so exp() sees
            # sane values (NaN/Inf notifications would distort timing)
            if lite:
                with tc.tile_pool(name="zf", bufs=2) as zf:
                    zt = zf.tile([128, S], F32, name="zt", tag="zt")
                    nc.vector.memset(zt[:], 0.0)
                    for t in range(8):
                        nc.sync.dma_start(xt.ap()[128 * t:128 * t + 128, :], zt[:])
                    for jn in range(16):
                        nc.sync.dma_start(xn.ap()[128 * jn:128 * jn + 128, :],
                                          zt[:, 0:E])
                    for t in range(8):
                        nc.sync.dma_start(wo.ap()[128 * t:128 * t + 128, :],
                                          zt[:, 0:E])

            # ================= phase 0: Wvo = blockdiag(Wv) @ Wo, bvWo+bo ====
            if phases >= 0:
              with (
                tc.tile_pool(name="ph0", bufs=2) as p0,
                tc.tile_pool(name="ps0a", bufs=1, space="PSUM") as ps0a,
                tc.tile_pool(name="ps0b", bufs=2, space="PSUM") as ps0b,
                tc.tile_pool(name="dr0", bufs=1, space="DRAM") as dr0,
            ):
               for rep in range(p0_reps):
                bvwo_ps = ps0a.tile([1, E], F32, name=f"bvwo_ps_{rep}", tag="bvwo")
                for t in range(8):
                    wo_f = p0.tile([128, E], F32, name=f"wof_{rep}_{t}", tag="wo_inf")
                    nc.sync.dma_start(wo_f[:], wo.ap()[128 * t:128 * t + 128, :])
                    wo_t = p0.tile([128, E], F32R, name=f"wo_{rep}_{t}", tag="wo_in")
                    nc.vector.tensor_copy(wo_t[:], wo_f[:])
                    for nt in range(2):
                        ns = slice(512 * nt, 512 * nt + 512)
                        # f32r matmuls cannot target col-offset 64, so use
                        # two 64-row psum tiles at base 0 (row tiling only)
                        wvpsA = ps0b.tile([64, 512], F32,
                                          name=f"wvpsA_{rep}_{t}_{nt}", tag="wvopsA")
                        nc.tensor.matmul(wvpsA[:], wvt2_t[0:64, :],
                                         wo_t[0:64, ns], start=True, stop=True)
                        wvpsB = ps0b.tile([64, 512], F32,
                                          name=f"wvpsB_{rep}_{t}_{nt}", tag="wvopsB")
                        nc.tensor.matmul(wvpsB[:], wvt2_t[64:128, :],
                                         wo_t[64:128, ns], start=True, stop=True)
                        nc.vector.tensor_copy(wvo[t][0:64, ns], wvpsA[:])
                        nc.vector.tensor_copy(wvo[t][64:128, ns], wvpsB[:])
                        nc.tensor.matmul(bvwo_ps[0:1, ns], bv2_t[:], wo_t[:, ns],
                                         start=(t == 0), stop=(t == 7))
                bosum_t = p0.tile([1, E], F32, name=f"bosum_{rep}", tag="bosum")
                nc.vector.tensor_add(bosum_t[:], bvwo_ps[:], borow_t[:])
                bod = dr0.tile([1, E], F32, name=f"bod_{rep}", tag="bod")
                nc.sync.dma_start(bod[:], bosum_t[:])
                nc.sync.dma_start(bobc_t[:], bod[:].broadcast_to([128, E]))

            # ================= phase 1: qpT / qpn (cos via Sin) ==============
            if phases >= 1:
              with tc.tile_pool(name="ph1", bufs=2) as p1:
               for rep in range(p1_reps):
                inv2pi = 1.0 / (2.0 * PI)
                for t in range(8):
                    xt_t = p1.tile([128, S], F32, name=f"xt_{rep}_{t}", tag="xt_in")
                    nc.sync.dma_start(xt_t[:], xt.ap()[128 * t:128 * t + 128, :])
                    # u = x/(2pi) + th'   (th' = (theta+pi/2)/2pi per-partition)
                    nc.vector.tensor_scalar(xt_t[:], xt_t[:], inv2pi,
                                            tht_t[:, 0:1], A.mult, A.add)
                    rt = p1.tile([128, S], F32, name=f"rt_{rep}_{t}", tag="rt")
                    nc.vector.tensor_scalar(rt[:], xt_t[:], MAGIC, MAGIC,
                                            A.add, A.subtract)
                    nc.vector.tensor_tensor(xt_t[:], xt_t[:], rt[:], A.subtract)
                    nc.scalar.activation(qpT[t][:], xt_t[:], AF.Sin,
                                         bias=0.0, scale=2.0 * PI)
                for jn in range(16):
                    xn_t = p1.tile([128, E], F32, name=f"xn_{rep}_{jn}", tag="xn_in")
                    nc.sync.dma_start(xn_t[:], xn.ap()[128 * jn:128 * jn + 128, :])
                    nc.vector.scalar_tensor_tensor(xn_t[:], xn_t[:], inv2pi,
                                                   thbc_t[:], A.mult, A.add)
                    tn = p1.tile([128, E], F32, name=f"tn_{rep}_{jn}", tag="tn")
                    nc.vector.tensor_scalar(tn[:], xn_t[:], MAGIC, MAGIC,
                                            A.add, A.subtract)
                    nc.vector.tensor_tensor(xn_t[:], xn_t[:], tn[:], A.subtract)
                    nc.vector.memset(qpn[jn][:], 1.0)
                    qv = qpn[jn][:].rearrange("p (h c) -> p h c", c=65)
                    uv = xn_t[:].rearrange("p (h c) -> p h c", c=64)
                    nc.scalar.activation(qv[:, :, 0:64], uv, AF.Sin,
                                         bias=0.0, scale=2.0 * PI)

            # ============ phase 2+3: projections + attention per pair ========
            if phases >= 2:
              with (
                tc.tile_pool(name="kq", bufs=2) as kq_pool,
                tc.tile_pool(name="et", bufs=2) as et_pool,
                tc.tile_pool(name="crw", bufs=6) as crw_pool,
                tc.tile_pool(name="nrm", bufs=3) as nrm_pool,
                tc.tile_pool(name="drb", bufs=6, space="DRAM") as dr_pool,
                tc.tile_pool(name="ps_s", bufs=3, space="PSUM") as ps_s,
                tc.tile_pool(name="ps_c", bufs=2, space="PSUM") as ps_c,
              ):
               for rep in range(attn_reps):
                # deferred normalization work from the previous pair: emitting
                # it here lets its DVE/DMA ops overlap this pair's attention
                pending = []

                def flush_pending():
                    for (tt, it_, head, craw) in pending:
                        isl_ = slice(512 * it_, 512 * it_ + 512)
                        sfx = f"{rep}_{tt}_{it_}_{head}"
                        # denominators -> DRAM -> reload spread over 64
                        # partitions so reciprocal uses 64 lanes, not 1
                        dr1 = dr_pool.tile([1, 512], F32,
                                           name=f"dr1_{sfx}", tag="dr1")
                        nc.sync.dma_start(dr1[:], craw[64:65, :])
                        den8 = nrm_pool.tile([64, 8], F32,
                                             name=f"den8_{sfx}", tag="den8")
                        nc.sync.dma_start(
                            den8[:],
                            dr1[:].rearrange("a (b c) -> (a b) c", c=8))
                        rec8 = nrm_pool.tile([64, 8], F32,
                                             name=f"rec8_{sfx}", tag="rec8")
                        nc.vector.reciprocal(rec8[:], den8[:])
                        dr2 = dr_pool.tile([1, 512], F32,
                                           name=f"dr2_{sfx}", tag="dr2")
                        nc.sync.dma_start(
                            dr2[:].rearrange("a (b c) -> (a b) c", c=8),
                            rec8[:])
                        bc = nrm_pool.tile([64, 512], F32,
                                           name=f"bc_{sfx}", tag="bc")
                        nc.sync.dma_start(bc[:], dr2[:].broadcast_to([64, 512]))
                        nc.vector.tensor_mul(
                            ctxT[tt][64 * head:64 * head + 64, isl_],
                            craw[0:64, :], bc[:])
                    pending.clear()

                for t in range(8):
                    hA, hB = 2 * t, 2 * t + 1
                    kT = kq_pool.tile([128, S], BF16, name=f"kT_{rep}_{t}", tag="kT")
                    qT = kq_pool.tile([128, SQ], BF16, name=f"qT_{rep}_{t}", tag="qT")
                    for st in range(4):
                        ss = slice(512 * st, 512 * st + 512)
                        pps = ps_s.tile([128, 512], F32,
                                        name=f"kps_{rep}_{t}_{st}", tag="spair")
                        nc.tensor.matmul(pps[0:64, :], wk2_t[0:64, :],
                                         qpT[t][0:64, ss], start=True, stop=True)
                        nc.tensor.matmul(pps[64:128, :], wk2_t[64:128, :],
                                         qpT[t][64:128, ss], start=True, stop=True)
                        nc.vector.tensor_scalar_add(kT[:, ss], pps[:],
                                                    bk2_t[:, 0:1])
                    for st in range(2):
                        ss = slice(512 * st, 512 * st + 512)
                        pps = ps_s.tile([128, 512], F32,
                                        name=f"qps_{rep}_{t}_{st}", tag="spair")
                        nc.tensor.matmul(pps[0:64, :], wq2_t[0:64, :],
                                         qpT[t][0:64, ss], start=True, stop=True)
                        nc.tensor.matmul(pps[64:128, :], wq2_t[64:128, :],
                                         qpT[t][64:128, ss], start=True, stop=True)
                        nc.vector.tensor_scalar_add(qT[:, ss], pps[:],
                                                    bq2_t[:, 0:1])
                    # previous pair's normalization drains into this pair's
                    # attention window
                    flush_pending()

                    for it in range(2):
                        isl = slice(512 * it, 512 * it + 512)
                        cA = ps_c.tile([65, 512], F32,
                                       name=f"cA_{rep}_{t}_{it}", tag="ctx")
                        cB = ps_c.tile([65, 512], F32,
                                       name=f"cB_{rep}_{t}_{it}", tag="ctx")
                        for j2 in range(8):
                            sA = ps_s.tile([128, 1024], F32,
                                           name=f"sA_{rep}_{t}_{it}_{j2}", tag="spair")
                            sB = ps_s.tile([128, 1024], F32,
                                           name=f"sB_{rep}_{t}_{it}_{j2}", tag="spair")
                            for hf in range(2):
                                jc = 2 * j2 + hf
                                js = slice(128 * jc, 128 * jc + 128)
                                hs = slice(512 * hf, 512 * hf + 512)
                                nc.tensor.matmul(sA[:, hs], kT[0:64, js],
                                                 qT[0:64, isl],
                                                 start=True, stop=True)
                                nc.tensor.matmul(sB[:, hs], kT[64:128, js],
                                                 qT[64:128, isl],
                                                 start=True, stop=True)
                            eA = et_pool.tile([128, 1024], BF16,
                                              name=f"eA_{rep}_{t}_{it}_{j2}", tag="eA")
                            nc.scalar.activation(eA[:], sA[:], AF.Exp,
                                                 bias=0.0, scale=0.125)
                            eB = et_pool.tile([128, 1024], BF16,
                                              name=f"eB_{rep}_{t}_{it}_{j2}", tag="eB")
                            nc.scalar.activation(eB[:], sB[:], AF.Exp,
                                                 bias=0.0, scale=0.125)
                            for hf in range(2):
                                jc = 2 * j2 + hf
                                hs = slice(512 * hf, 512 * hf + 512)
                                st_ = (j2 == 0 and hf == 0)
                                sp_ = (j2 == 7 and hf == 1)
                                nc.tensor.matmul(
                                    cA[:], qpn[jc][:, 65 * hA:65 * hA + 65],
                                    eA[:, hs], start=st_, stop=sp_)
                                nc.tensor.matmul(
                                    cB[:], qpn[jc][:, 65 * hB:65 * hB + 65],
                                    eB[:, hs], start=st_, stop=sp_)
                        # free the ctx psum banks immediately; normalization
                        # is deferred to the next pair
                        for head, cps in ((0, cA), (1, cB)):
                            craw = crw_pool.tile(
                                [65, 512], F32,
                                name=f"craw_{rep}_{t}_{it}_{head}", tag="craw")
                            nc.vector.tensor_copy(craw[:], cps[:])
                            pending.append((t, it, head, craw))
                flush_pending()

            # ================= phase 4: out projection =======================
            if phases >= 4:
              with (
                tc.tile_pool(name="ph4", bufs=2) as p4,
                tc.tile_pool(name="ps4", bufs=2, space="PSUM") as ps4,
            ):
               for rep in range(p4_reps):
                for ic in range(8):
                    ics = slice(128 * ic, 128 * ic + 128)
                    ot = p4.tile([128, E], F32, name=f"ot_{rep}_{ic}", tag="ot")
                    for nt in range(2):
                        ns = slice(512 * nt, 512 * nt + 512)
                        ops_ = ps4.tile([128, 512], F32,
                                        name=f"ops_{rep}_{ic}_{nt}", tag="ops")
                        for t in range(8):
                            nc.tensor.matmul(ops_[:], ctxT[t][:, ics],
                                             wvo[t][:, ns],
                                             start=(t == 0), stop=(t == 7))
                        nc.vector.tensor_add(ot[:, ns], ops_[:], bobc_t[:, ns])
                    nc.sync.dma_start(out.ap()[ics, :], ot[:])

    return nc


def kernel(x, theta, Wq, bq, Wk, bk, Wv, bv, Wo, bo):
    x = np.asarray(x, np.float32)
    theta = np.asarray(theta, np.float32)
    Wq = np.asarray(Wq, np.float32)
    Wk = np.asarray(Wk, np.float32)
    Wv = np.asarray(Wv, np.float32)
    Wo = np.asarray(Wo, np.float32)
    bq = np.asarray(bq, np.float32)
    bk = np.asarray(bk, np.float32)
    bv = np.asarray(bv, np.float32)
    bo = np.asarray(bo, np.float32)

    nc = _build()
    _split_multiwaits(nc)

    th2 = np.concatenate([theta, theta]).reshape(128, 1)
    tht = ((th2 + PI / 2) / (2 * PI)).astype(np.float32)
    thbc = np.tile(
        ((np.tile(theta, H) + PI / 2) / (2 * PI)).astype(np.float32).reshape(1, E),
        (128, 1),
    )
    wq2 = np.concatenate([Wq, Wq], axis=0).astype(nbf16)
    wk2 = np.concatenate([Wk, Wk], axis=0).astype(nbf16)
    wvt2 = np.ascontiguousarray(np.concatenate([Wv.T, Wv.T], axis=0), dtype=np.float32)
    wo_bf = np.ascontiguousarray(Wo, dtype=np.float32)
    bq2 = np.concatenate([bq, bq]).reshape(128, 1).astype(np.float32)
    bk2 = np.concatenate([bk, bk]).reshape(128, 1).astype(np.float32)
    bv2 = np.concatenate([bv, bv]).reshape(128, 1).astype(np.float32)
    bo_r = bo.reshape(1, E).astype(np.float32)

    in_maps = []
    for c in range(N_CORES):
        b, j = c // 2, c % 2
        xb = np.roll(x[b], -SQ * j, axis=0)
        in_maps.append(dict(
            xt=np.ascontiguousarray(xb.T),
            xn=np.ascontiguousarray(xb),
            tht=tht, thbc=thbc, wq2=wq2, wk2=wk2, wvt2=wvt2, wo=wo_bf,
            bq2=bq2, bk2=bk2, bv2=bv2, bo_r=bo_r,
        ))

    kw = {}
    if TRACE:
        kw = dict(trace=True, trace_cores=[0])
    res = run_bass_kernel_spmd(nc, in_maps, core_ids=list(range(N_CORES)), **kw)
    global LAST_RES
    LAST_RES = res

    out = np.empty((B, S, E), np.float32)
    for c in range(N_CORES):
        b, j = c // 2, c % 2
        out[b, SQ * j:SQ * (j + 1), :] = res.results[c]["out"]
    return out
